# revision 1
# baseline (speedup 1.0000x reference)
"""Trainium2 Bass kernel for nn_FRAMES_VisionTransformer_28166395527587.

The reference computation (drop CLS token -> 1D nearest resize 768->729 ->
reverse-patching reshape to (144,126,126) -> 3D nearest resize to (64,64,64))
is a pure gather with compile-time-constant index maps:

    out[b, 0, z, y, x] = hs[b, 1 + 196*(z//4) + 14*r(y) + p(x),
                            f[81*d0(z) + 9*d1(y) + d2(x)]]

with  d0(z) = [0,2,4,6][z%4],          i(z) = z//4
      c(y)  = floor(63y/32) = 9*r + d1  (0, then odds 1..63, evens 64..124)
      c(x)  = floor(63x/32) = 9*p + d2  (same map)
      f[j]  = floor32(j*768/729)        (float32 floor, matching jax)

Sharding: pure data parallel, 8 batch samples per core.  The CLS token is
stripped host-side so the (sample, couple) block stride is uniform; on each
core the 128 SBUF partitions then hold the 128 (sample, couple) blocks and
every DMA spans all 128 partitions (all 16 SDMA engines).  The gather runs
as a short sequence of strided on-chip copies shared by all partitions.

Work is split into 16 sub-rounds (q, h, token-row half); the two halves of
each round load through different DMA queues (HWDGE + SWDGE) concurrently.
"""

import numpy as np

# ---------------------------------------------------------------- constants
B_FULL = 64
N_CORES = 8
B_CORE = B_FULL // N_CORES  # 8 samples per core


def _nearest_f32(out_size, in_size):
    """float32-exact emulation of the reference's jnp _nearest_idx.

    jax computes floor(arange(out) * (in/out)) in float32; at j=486 the
    product rounds to 511.999... so floor gives 511, not the exact 512."""
    ratio = np.float32(in_size / out_size)
    j = np.arange(out_size, dtype=np.int32).astype(np.float32)
    return np.floor((j * ratio).astype(np.float32)).astype(np.int64)


_f = _nearest_f32(729, 768)  # feature resize map
_c = _nearest_f32(64, 126)  # y/x resize map (= 9*r + d1)

DZ = [0, 2, 4, 6]  # d0 values for z%4
LOS = [int(_f[81 * d0]) for d0 in DZ]  # [0, 170, 341, 511]
UW = 96  # features per token: 384 B = 3x128 B aligned descriptors

# token-row split of each (q, h) round: rows 0-2 -> yl 0-13, rows 3-6 -> 14-31
SUBS = [(0, 3, 0, 14), (3, 4, 14, 18)]  # (rt0, nrt, yl0, nyl)


def _feat_runs(q):
    """Contiguous runs of the 81-feature selection for d0-slice q.

    Returns [(j0, n, u0)]: M[:, j0:j0+n] = L[:, u0:u0+n]."""
    g = _f[81 * DZ[q] + np.arange(81)] - LOS[q]
    runs, start = [], 0
    for k in range(1, 81):
        if g[k] != g[k - 1] + 1:
            runs.append((start, k - start, int(g[start])))
            start = k
    runs.append((start, 81 - start, int(g[start])))
    return runs


FEAT_RUNS = [_feat_runs(q) for q in range(4)]


def _x_runs():
    """x-gather runs: [(p, x0, nx, d20)] with d2 = d20+2k, x = x0+k."""
    runs, x = [], 0
    while x < 64:
        p, d20 = int(_c[x]) // 9, int(_c[x]) % 9
        n = 1
        while x + n < 64 and _c[x + n] == _c[x] + 2 * n and _c[x + n] // 9 == p:
            n += 1
        runs.append((p, x, n, d20))
        x += n
    return runs


X_RUNS = _x_runs()


def _y_runs(rt0, nrt):
    """y-gather runs for rows [rt0, rt0+nrt): [(yl0, n, c0)] with
    O[yl0+k] = X[c0+2k] in the sub-round's local c coordinates.

    The local map cl(yl) = c(32h+yl) - 63h is identical for both h."""
    cl = [int(_c[yl]) for yl in range(32)]  # h=0 local map: [0,1,3,...,61]
    lo, hi = 9 * rt0, 9 * (rt0 + nrt)
    yls = [yl for yl in range(32) if lo <= cl[yl] < hi]
    runs, i = [], 0
    while i < len(yls):
        y0, n = yls[i], 1
        while (
            i + n < len(yls)
            and yls[i + n] == y0 + n
            and cl[y0 + n] == cl[y0] + 2 * n
        ):
            n += 1
        runs.append((y0, n, cl[y0] - lo))
        i += n
    return runs


Y_RUNS = [_y_runs(rt0, nrt) for (rt0, nrt, _, _) in SUBS]

# ------------------------------------------------------------- bass program
_NC_CACHE = None


def _build_nc():
    import concourse.bacc as bacc
    import concourse.tile as tile
    from concourse import mybir

    nc = bacc.Bacc(None, target_bir_lowering=False, debug=False)
    f32 = mybir.dt.float32

    # CLS token already stripped host-side -> uniform (b, i) block stride.
    hs = nc.dram_tensor("hs", (B_CORE, 3136, 768), f32, kind="ExternalInput")
    out = nc.dram_tensor("out", (B_CORE, 1, 64, 64, 64), f32, kind="ExternalOutput")

    # [(b i), t, u]: 128 blocks x 196 tokens x feature
    hs_v = hs.ap().rearrange("b (i t) u -> (b i) t u", i=16)
    # [(b i), q, (y x)]: z = 4i+q; y,x merge into one contiguous dim
    out_v = out.ap().rearrange(
        "b c (i q) y x -> (b i) c q (y x)", i=16, q=4
    )

    with tile.TileContext(nc) as tc:
        with (
            tc.tile_pool(name="lp", bufs=5) as lp,
            tc.tile_pool(name="mp", bufs=3) as mp,
            tc.tile_pool(name="xp", bufs=1) as xp,
            tc.tile_pool(name="op", bufs=2) as op,
        ):
            s = 0
            for q in range(4):
                for h in range(2):
                    for sub, (rt0, nrt, yl0, nyl) in enumerate(SUBS):
                        nt = 14 * nrt  # tokens in this sub-round
                        t0 = 98 * h + 14 * rt0
                        lo = LOS[q]

                        # ---- load [128, nt, UW]; halves go to different
                        # DMA queues (HWDGE / SWDGE) and run concurrently
                        # payload 88 floats (352 B) per token, dst stride 96
                        # (384 B) keeps every SBUF write 128 B aligned
                        L = lp.tile([128, 56 * UW], f32, tag="L")
                        eng = nc.sync if s % 2 == 0 else nc.gpsimd
                        L3 = L[:, : nt * UW].rearrange("p (t u) -> p t u", u=UW)
                        eng.dma_start(
                            out=L3[:, :, :88], in_=hs_v[:, t0 : t0 + nt, lo : lo + 88]
                        )

                        # ---- feature compaction -> [part, nt, 81] (ScalarE)
                        M = mp.tile([128, 56 * 81], f32, tag="M")
                        M3 = M[:, : nt * 81].rearrange("p (t j) -> p t j", j=81)
                        for (j0, n, u0) in FEAT_RUNS[q]:
                            nc.scalar.copy(
                                out=M3[:, :, j0 : j0 + n],
                                in_=L3[:, :, u0 : u0 + n],
                            )

                        # ---- x-gather -> [part, rt, d1, x] (VectorE)
                        X = xp.tile([128, 4 * 9 * 64], f32, tag="X")
                        M5 = M[:, : nt * 81].rearrange(
                            "p (rt pp d1 d2) -> p rt pp d1 d2",
                            rt=nrt, pp=14, d1=9,
                        )
                        X4 = X[:, : nrt * 9 * 64].rearrange(
                            "p (rt d1 x) -> p rt d1 x", rt=nrt, x=64
                        )
                        for (pp, x0, nx, d20) in X_RUNS:
                            nc.vector.tensor_copy(
                                out=X4[:, :, :, x0 : x0 + nx],
                                in_=M5[:, :, pp, :, d20 : d20 + 2 * nx - 1 : 2],
                            )

                        # ---- y-gather -> [part, nyl, 64] (VectorE)
                        O = op.tile([128, 18 * 64], f32, tag="O")
                        X3 = X[:, : nrt * 9 * 64].rearrange(
                            "p (cl x) -> p cl x", x=64
                        )
                        O3 = O[:, : nyl * 64].rearrange(
                            "p (yl x) -> p yl x", x=64
                        )
                        for (y0, n, c0) in Y_RUNS[sub]:
                            nc.vector.tensor_copy(
                                out=O3[:, y0 - yl0 : y0 - yl0 + n, :],
                                in_=X3[:, c0 : c0 + 2 * n - 1 : 2, :],
                            )

                        # ---- store: contiguous (y-range x 64x) per partition
                        ybase = (32 * h + yl0) * 64
                        nc.scalar.dma_start(
                            out=out_v[:, 0, q, ybase : ybase + nyl * 64],
                            in_=O[:, : nyl * 64],
                        )
                        s += 1

    nc.compile()
    return nc


def _get_nc():
    global _NC_CACHE
    if _NC_CACHE is None:
        _NC_CACHE = _build_nc()
    return _NC_CACHE


# ------------------------------------------------------------------ runner
def _in_maps(hidden_states: np.ndarray) -> list:
    hs = np.asarray(hidden_states, dtype=np.float32)
    assert hs.shape == (B_FULL, 3137, 768), hs.shape
    return [
        {"hs": np.ascontiguousarray(hs[c * B_CORE : (c + 1) * B_CORE, 1:, :])}
        for c in range(N_CORES)
    ]


def kernel(hidden_states: np.ndarray) -> np.ndarray:
    import time

    from concourse import bass_utils

    nc = _get_nc()
    in_maps = _in_maps(hidden_states)
    last_err = None
    for attempt in range(3):
        try:
            res = bass_utils.run_bass_kernel_spmd(
                nc, in_maps, core_ids=list(range(N_CORES))
            )
            return np.concatenate([r["out"] for r in res.results], axis=0)
        except Exception as e:  # transient device hiccups self-heal in ~1 min
            last_err = e
            time.sleep(45 * (attempt + 1))
    raise last_err



# revision 3
# speedup vs baseline: 1.0796x; 1.0796x over previous
"""Trainium2 Bass kernel for nn_FRAMES_VisionTransformer_28166395527587.

The reference computation (drop CLS token -> 1D nearest resize 768->729 ->
reverse-patching reshape to (144,126,126) -> 3D nearest resize to (64,64,64))
is a pure gather with compile-time-constant index maps:

    out[b, 0, z, y, x] = hs[b, 1 + 196*(z//4) + 14*r + p, f[81*d0 + 9*d1 + d2]]

with  d0 = [0,2,4,6][z%4], i = z//4, c(y) = floor32(63y/32) = 9r + d1,
      c(x) = 9p + d2, f = float32-exact floor(arange(729) * 768/729).

Key structure exploited here: the 64 selected c-values of 126 are
{0} + odds 1..61  (+63 offset for the upper half), so per token row r the
needed d1 set alternates between the odd {1,3,5,7} and even {0,2,4,6,8}
parity classes, with uniform stride-2 row patterns.  This lets the kernel:

  * load, per (q, h) round, only the per-row-class feature windows
    (rows 0,2,4,6 need d1<=7 -> ~75 floats/token; rows 1,3,5 need d1<=8
    -> ~85 floats/token) instead of a uniform 88-float window;
  * fold the y-resize row-selection directly into the feature-compaction
    copies (writing only the 32 needed c-rows of 63), roughly halving
    on-chip copy traffic vs compact-everything-then-select.

Sharding: pure data parallel, 8 batch samples per core.  CLS stripped
host-side so the 128 SBUF partitions hold the 128 (sample, couple) blocks.
Per (q, h) round: 2 DMA loads (HWDGE + SWDGE in parallel), ~15 compaction
copies on one compute engine, 6 x-gather copies on the other (engines
alternate per round), one contiguous 8KB/partition store.
"""

import numpy as np

# ---------------------------------------------------------------- constants
B_FULL = 64
N_CORES = 8
B_CORE = B_FULL // N_CORES  # 8 samples per core


def _nearest_f32(out_size, in_size):
    """float32-exact emulation of the reference's jnp _nearest_idx.

    jax computes floor(arange(out) * (in/out)) in float32; at j=486 the
    product rounds to 511.999... so floor gives 511, not the exact 512."""
    ratio = np.float32(in_size / out_size)
    j = np.arange(out_size, dtype=np.int32).astype(np.float32)
    return np.floor((j * ratio).astype(np.float32)).astype(np.int64)


_f = _nearest_f32(729, 768)  # feature resize map
DZ = [0, 2, 4, 6]  # d0 values for z%4
LOS = [int(_f[81 * d0]) for d0 in DZ]


def _runs(vals):
    """Contiguous runs of an int sequence: [(start_idx, length)]."""
    runs, s = [], 0
    for k in range(1, len(vals)):
        if vals[k] != vals[k - 1] + 1:
            runs.append((s, k - s))
            s = k
    runs.append((s, len(vals) - s))
    return runs


def _compact_specs(q):
    """[(tile, nk, yl0, d2_0, n, u0)]: M[yl0+9k, p, d2_0:+n] = tile[k, p, u0:+n].

    tile 'A': LA rows 0,2,4,6; 'B': LB rows 1,3,5; 'A0': LA row 0 only."""
    g = (_f[81 * DZ[q] + np.arange(81)] - LOS[q]).astype(int)
    specs = []
    for d1 in (1, 3, 5, 7):  # odd d1 -> rows 0,2,4,6
        for (s, n) in _runs(g[9 * d1 : 9 * d1 + 9]):
            specs.append(("A", 4, (d1 + 1) // 2, s, n, int(g[9 * d1 + s])))
    for d1 in (2, 4, 6, 8):  # even d1 -> rows 1,3,5
        for (s, n) in _runs(g[9 * d1 : 9 * d1 + 9]):
            specs.append(("B", 3, 5 + d1 // 2, s, n, int(g[9 * d1 + s])))
    for (s, n) in _runs(g[0:9]):  # d1 = 0 main -> rows 1,3,5 (c = 9, 27, 45)
        specs.append(("B", 3, 5, s, n, int(g[s])))
    for (s, n) in _runs(g[0:9]):  # d1 = 0 special -> row 0 (c = 0 / 63)
        specs.append(("A0", 1, 0, s, n, int(g[s])))
    return specs, int(g[71]) + 1, int(g[80]) + 1  # specs, wA, wB


CSPECS = [_compact_specs(q) for q in range(4)]

# x-gather families: O[yl, ob+9g+okoff+k] = M[yl, ib+18g+ikoff+2k], k < nx
# (base shifts keep every rearrange-window inside the real 64/126 extents)
XFAM = [
    (1, 4, 0, 4, 1, 0),
    (5, 3, 0, 5, 9, 0),
    (28, 4, 5, 4, 54, 10),
    (37, 3, 0, 5, 72, 0),
]
XSINGLE = [(0, 0), (32, 63)]  # (x, c) singletons

# ------------------------------------------------------------- bass program
_NC_CACHE = None


def _build_nc():
    import concourse.bacc as bacc
    import concourse.tile as tile
    from concourse import mybir

    nc = bacc.Bacc(None, target_bir_lowering=False, debug=False)
    f32 = mybir.dt.float32

    # CLS token already stripped host-side -> uniform (b, i) block stride.
    hs = nc.dram_tensor("hs", (B_CORE, 3136, 768), f32, kind="ExternalInput")
    out = nc.dram_tensor("out", (B_CORE, 1, 64, 64, 64), f32, kind="ExternalOutput")

    # [(b i), t, u]: 128 blocks x 196 tokens x feature
    hs_v = hs.ap().rearrange("b (i t) u -> (b i) t u", i=16)
    # [(b i), q, (y x)]: z = 4i+q; y,x merge into one contiguous dim
    out_v = out.ap().rearrange("b c (i q) y x -> (b i) c q (y x)", i=16, q=4)

    with tile.TileContext(nc) as tc:
        with (
            tc.tile_pool(name="lpa", bufs=3) as lpa,
            tc.tile_pool(name="lpb", bufs=3) as lpb,
            tc.tile_pool(name="mp", bufs=3) as mp,
            tc.tile_pool(name="op", bufs=2) as op,
        ):
            rnd = 0
            for q in range(4):
                specs, wA, wB = CSPECS[q]
                lo = LOS[q]
                for h in range(2):
                    t0 = 98 * h
                    # ---- loads: rows 0,2,4,6 and rows 1,3,5 on separate
                    # DGE paths (HWDGE sync / SWDGE gpsimd), in parallel.
                    # DMA APs allow at most partition+2 dims, so each
                    # stride-2 row goes as its own [128, 14, w] transfer.
                    LA = lpa.tile([128, 4 * 14 * wA], f32, tag="LA")
                    LA4 = LA.rearrange("p (k t u) -> p k t u", k=4, u=wA)
                    for k in range(4):
                        ts = t0 + 28 * k
                        nc.sync.dma_start(
                            out=LA4[:, k], in_=hs_v[:, ts : ts + 14, lo : lo + wA]
                        )

                    LB = lpb.tile([128, 3 * 14 * wB], f32, tag="LB")
                    LB4 = LB.rearrange("p (k t u) -> p k t u", k=3, u=wB)
                    for k in range(3):
                        ts = t0 + 28 * k + 14
                        nc.gpsimd.dma_start(
                            out=LB4[:, k], in_=hs_v[:, ts : ts + 14, lo : lo + wB]
                        )

                    # ---- compact + y-select -> M [32 yl, 126 c]; the
                    # compaction and x-gather alternate engines per round
                    ce = nc.scalar if rnd % 2 == 0 else nc.vector
                    xe = nc.vector if rnd % 2 == 0 else nc.scalar
                    M = mp.tile([128, 32 * 126], f32, tag="M")
                    M4 = M.rearrange("p (yl t d2) -> p yl t d2", yl=32, d2=9)
                    for (tl, nk, yl0, d2_0, n, u0) in specs:
                        src4 = LA4 if tl in ("A", "A0") else LB4
                        if tl == "A0":
                            src4 = src4[:, 0:1]
                        dst = M4[:, yl0 : yl0 + 9 * (nk - 1) + 1 : 9, :, d2_0 : d2_0 + n]
                        if ce is nc.scalar:
                            ce.copy(out=dst, in_=src4[:, :, :, u0 : u0 + n])
                        else:
                            ce.tensor_copy(out=dst, in_=src4[:, :, :, u0 : u0 + n])

                    # ---- x-gather -> O [32 yl, 64 x]
                    O = op.tile([128, 32 * 64], f32, tag="O")
                    O3 = O.rearrange("p (yl x) -> p yl x", x=64)
                    M3 = M.rearrange("p (yl c) -> p yl c", c=126)
                    for (x, c) in XSINGLE:
                        dst, s2 = O3[:, :, x : x + 1], M3[:, :, c : c + 1]
                        if xe is nc.scalar:
                            xe.copy(out=dst, in_=s2)
                        else:
                            xe.tensor_copy(out=dst, in_=s2)
                    for (ob, og, okoff, nx, ib, ikoff) in XFAM:
                        o4 = O3[:, :, ob : ob + 9 * og].rearrange(
                            "p yl (g k) -> p yl g k", g=og
                        )[:, :, :, okoff : okoff + nx]
                        i4 = M3[:, :, ib : ib + 18 * og].rearrange(
                            "p yl (g c) -> p yl g c", g=og
                        )[:, :, :, ikoff : ikoff + 2 * nx - 1 : 2]
                        if xe is nc.scalar:
                            xe.copy(out=o4, in_=i4)
                        else:
                            xe.tensor_copy(out=o4, in_=i4)

                    # ---- store: contiguous 8 KB per partition
                    nc.scalar.dma_start(
                        out=out_v[:, 0, q, 2048 * h : 2048 * (h + 1)],
                        in_=O[:, :],
                    )
                    rnd += 1

    nc.compile()
    return nc


def _get_nc():
    global _NC_CACHE
    if _NC_CACHE is None:
        _NC_CACHE = _build_nc()
    return _NC_CACHE


# ------------------------------------------------------------------ runner
def _in_maps(hidden_states: np.ndarray) -> list:
    hs = np.asarray(hidden_states, dtype=np.float32)
    assert hs.shape == (B_FULL, 3137, 768), hs.shape
    return [
        {"hs": np.ascontiguousarray(hs[c * B_CORE : (c + 1) * B_CORE, 1:, :])}
        for c in range(N_CORES)
    ]


def kernel(hidden_states: np.ndarray) -> np.ndarray:
    import time

    from concourse import bass_utils

    nc = _get_nc()
    in_maps = _in_maps(hidden_states)
    last_err = None
    for attempt in range(3):
        try:
            res = bass_utils.run_bass_kernel_spmd(
                nc, in_maps, core_ids=list(range(N_CORES))
            )
            return np.concatenate([r["out"] for r in res.results], axis=0)
        except Exception as e:  # transient device hiccups self-heal in ~1 min
            last_err = e
            time.sleep(45 * (attempt + 1))
    raise last_err


# revision 4
# speedup vs baseline: 1.1191x; 1.0366x over previous
"""Trainium2 Bass kernel for nn_FRAMES_VisionTransformer_28166395527587.

The reference computation (drop CLS token -> 1D nearest resize 768->729 ->
reverse-patching reshape to (144,126,126) -> 3D nearest resize to (64,64,64))
is a pure gather with compile-time-constant index maps:

    out[b, 0, z, y, x] = hs[b, 1 + 196*(z//4) + 14*r + p, f[81*d0 + 9*d1 + d2]]

with  d0 = [0,2,4,6][z%4], i = z//4, c(y) = floor32(63y/32) = 9r + d1,
      c(x) = 9p + d2, f = float32-exact floor(arange(729) * 768/729).

Key structure exploited here: the 64 selected c-values of 126 are
{0} + odds 1..61  (+63 offset for the upper half), so per token row r the
needed d1 set alternates between the odd {1,3,5,7} and even {0,2,4,6,8}
parity classes, with uniform stride-2 row patterns.  This lets the kernel:

  * load, per (q, h) round, only the per-row-class feature windows
    (rows 0,2,4,6 need d1<=7 -> ~75 floats/token; rows 1,3,5 need d1<=8
    -> ~85 floats/token) instead of a uniform 88-float window;
  * fold the y-resize row-selection directly into the feature-compaction
    copies (writing only the 32 needed c-rows of 63), roughly halving
    on-chip copy traffic vs compact-everything-then-select.

Sharding: pure data parallel, 8 batch samples per core.  CLS stripped
host-side so the 128 SBUF partitions hold the 128 (sample, couple) blocks.
Per (q, h) round: 2 DMA loads (HWDGE + SWDGE in parallel), ~15 compaction
copies on one compute engine, 6 x-gather copies on the other (engines
alternate per round), one contiguous 8KB/partition store.
"""

import numpy as np

# ---------------------------------------------------------------- constants
B_FULL = 64
N_CORES = 8
B_CORE = B_FULL // N_CORES  # 8 samples per core


def _nearest_f32(out_size, in_size):
    """float32-exact emulation of the reference's jnp _nearest_idx.

    jax computes floor(arange(out) * (in/out)) in float32; at j=486 the
    product rounds to 511.999... so floor gives 511, not the exact 512."""
    ratio = np.float32(in_size / out_size)
    j = np.arange(out_size, dtype=np.int32).astype(np.float32)
    return np.floor((j * ratio).astype(np.float32)).astype(np.int64)


_f = _nearest_f32(729, 768)  # feature resize map
DZ = [0, 2, 4, 6]  # d0 values for z%4
LOS = [int(_f[81 * d0]) for d0 in DZ]


def _runs(vals):
    """Contiguous runs of an int sequence: [(start_idx, length)]."""
    runs, s = [], 0
    for k in range(1, len(vals)):
        if vals[k] != vals[k - 1] + 1:
            runs.append((s, k - s))
            s = k
    runs.append((s, len(vals) - s))
    return runs


def _compact_specs(q):
    """[(tile, nk, yl0, d2_0, n, u0)]: M[yl0+9k, p, d2_0:+n] = tile[k, p, u0:+n].

    tile 'A': LA rows 0,2,4,6; 'B': LB rows 1,3,5; 'A0': LA row 0 only."""
    g = (_f[81 * DZ[q] + np.arange(81)] - LOS[q]).astype(int)
    specs = []
    for d1 in (1, 3, 5, 7):  # odd d1 -> rows 0,2,4,6
        for (s, n) in _runs(g[9 * d1 : 9 * d1 + 9]):
            specs.append(("A", 4, (d1 + 1) // 2, s, n, int(g[9 * d1 + s])))
    for d1 in (2, 4, 6, 8):  # even d1 -> rows 1,3,5
        for (s, n) in _runs(g[9 * d1 : 9 * d1 + 9]):
            specs.append(("B", 3, 5 + d1 // 2, s, n, int(g[9 * d1 + s])))
    for (s, n) in _runs(g[0:9]):  # d1 = 0 main -> rows 1,3,5 (c = 9, 27, 45)
        specs.append(("B", 3, 5, s, n, int(g[s])))
    for (s, n) in _runs(g[0:9]):  # d1 = 0 special -> row 0 (c = 0 / 63)
        specs.append(("A0", 1, 0, s, n, int(g[s])))
    # pad windows to whole 32B beats so every SBUF write is beat-aligned
    # with no read-modify-write partials (payload == stride == 8k floats)
    wA = -(-(int(g[71]) + 1) // 8) * 8
    wB = -(-(int(g[80]) + 1) // 8) * 8
    return specs, wA, wB


CSPECS = [_compact_specs(q) for q in range(4)]

# x-gather families: O[yl, ob+9g+okoff+k] = M[yl, ib+18g+ikoff+2k], k < nx
# (base shifts keep every rearrange-window inside the real 64/126 extents)
XFAM = [
    (1, 4, 0, 4, 1, 0),
    (5, 3, 0, 5, 9, 0),
    (28, 4, 5, 4, 54, 10),
    (37, 3, 0, 5, 72, 0),
]
XSINGLE = [(0, 0), (32, 63)]  # (x, c) singletons

# ------------------------------------------------------------- bass program
_NC_CACHE = None


def _build_nc():
    import concourse.bacc as bacc
    import concourse.tile as tile
    from concourse import mybir

    nc = bacc.Bacc(None, target_bir_lowering=False, debug=False)
    f32 = mybir.dt.float32

    # CLS token already stripped host-side -> uniform (b, i) block stride.
    hs = nc.dram_tensor("hs", (B_CORE, 3136, 768), f32, kind="ExternalInput")
    out = nc.dram_tensor("out", (B_CORE, 1, 64, 64, 64), f32, kind="ExternalOutput")

    # [(b i), t, u]: 128 blocks x 196 tokens x feature
    hs_v = hs.ap().rearrange("b (i t) u -> (b i) t u", i=16)
    # [(b i), q, (y x)]: z = 4i+q; y,x merge into one contiguous dim
    out_v = out.ap().rearrange("b c (i q) y x -> (b i) c q (y x)", i=16, q=4)

    with tile.TileContext(nc) as tc:
        with (
            tc.tile_pool(name="lpa", bufs=3) as lpa,
            tc.tile_pool(name="lpb", bufs=3) as lpb,
            tc.tile_pool(name="mp", bufs=3) as mp,
            tc.tile_pool(name="op", bufs=2) as op,
        ):
            rnd = 0
            for q in range(4):
                specs, wA, wB = CSPECS[q]
                lo = LOS[q]
                for h in range(2):
                    t0 = 98 * h
                    # ---- loads: rows 0,2,4,6 and rows 1,3,5 on separate
                    # DGE paths (HWDGE sync / SWDGE gpsimd), in parallel.
                    # DMA APs allow at most partition+2 dims, so each
                    # stride-2 row goes as its own [128, 14, w] transfer.
                    LA = lpa.tile([128, 4 * 14 * wA], f32, tag="LA")
                    LA4 = LA.rearrange("p (k t u) -> p k t u", k=4, u=wA)
                    for k in range(4):
                        ts = t0 + 28 * k
                        nc.sync.dma_start(
                            out=LA4[:, k], in_=hs_v[:, ts : ts + 14, lo : lo + wA]
                        )

                    LB = lpb.tile([128, 3 * 14 * wB], f32, tag="LB")
                    LB4 = LB.rearrange("p (k t u) -> p k t u", k=3, u=wB)
                    for k in range(3):
                        ts = t0 + 28 * k + 14
                        nc.gpsimd.dma_start(
                            out=LB4[:, k], in_=hs_v[:, ts : ts + 14, lo : lo + wB]
                        )

                    # ---- compact + y-select -> M [32 yl, 126 c]; the
                    # compaction and x-gather alternate engines per round
                    ce = nc.scalar if rnd % 2 == 0 else nc.vector
                    xe = nc.vector if rnd % 2 == 0 else nc.scalar
                    M = mp.tile([128, 32 * 126], f32, tag="M")
                    M4 = M.rearrange("p (yl t d2) -> p yl t d2", yl=32, d2=9)
                    for (tl, nk, yl0, d2_0, n, u0) in specs:
                        src4 = LA4 if tl in ("A", "A0") else LB4
                        if tl == "A0":
                            src4 = src4[:, 0:1]
                        dst = M4[:, yl0 : yl0 + 9 * (nk - 1) + 1 : 9, :, d2_0 : d2_0 + n]
                        if ce is nc.scalar:
                            ce.copy(out=dst, in_=src4[:, :, :, u0 : u0 + n])
                        else:
                            ce.tensor_copy(out=dst, in_=src4[:, :, :, u0 : u0 + n])

                    # ---- x-gather -> O [32 yl, 64 x]
                    O = op.tile([128, 32 * 64], f32, tag="O")
                    O3 = O.rearrange("p (yl x) -> p yl x", x=64)
                    M3 = M.rearrange("p (yl c) -> p yl c", c=126)
                    for (x, c) in XSINGLE:
                        dst, s2 = O3[:, :, x : x + 1], M3[:, :, c : c + 1]
                        if xe is nc.scalar:
                            xe.copy(out=dst, in_=s2)
                        else:
                            xe.tensor_copy(out=dst, in_=s2)
                    for (ob, og, okoff, nx, ib, ikoff) in XFAM:
                        o4 = O3[:, :, ob : ob + 9 * og].rearrange(
                            "p yl (g k) -> p yl g k", g=og
                        )[:, :, :, okoff : okoff + nx]
                        i4 = M3[:, :, ib : ib + 18 * og].rearrange(
                            "p yl (g c) -> p yl g c", g=og
                        )[:, :, :, ikoff : ikoff + 2 * nx - 1 : 2]
                        if xe is nc.scalar:
                            xe.copy(out=o4, in_=i4)
                        else:
                            xe.tensor_copy(out=o4, in_=i4)

                    # ---- store: contiguous 8 KB per partition
                    nc.scalar.dma_start(
                        out=out_v[:, 0, q, 2048 * h : 2048 * (h + 1)],
                        in_=O[:, :],
                    )
                    rnd += 1

    nc.compile()
    return nc


def _get_nc():
    global _NC_CACHE
    if _NC_CACHE is None:
        _NC_CACHE = _build_nc()
    return _NC_CACHE


# ------------------------------------------------------------------ runner
def _in_maps(hidden_states: np.ndarray) -> list:
    hs = np.asarray(hidden_states, dtype=np.float32)
    assert hs.shape == (B_FULL, 3137, 768), hs.shape
    return [
        {"hs": np.ascontiguousarray(hs[c * B_CORE : (c + 1) * B_CORE, 1:, :])}
        for c in range(N_CORES)
    ]


def kernel(hidden_states: np.ndarray) -> np.ndarray:
    import time

    from concourse import bass_utils

    nc = _get_nc()
    in_maps = _in_maps(hidden_states)
    last_err = None
    for attempt in range(3):
        try:
            res = bass_utils.run_bass_kernel_spmd(
                nc, in_maps, core_ids=list(range(N_CORES))
            )
            return np.concatenate([r["out"] for r in res.results], axis=0)
        except Exception as e:  # transient device hiccups self-heal in ~1 min
            last_err = e
            time.sleep(45 * (attempt + 1))
    raise last_err
